# revision 2
# baseline (speedup 1.0000x reference)
"""Trainium2 Bass kernel v2 for nn_Conv1d_NN (kNN + strided conv).

Per batch b (2 per core, 8 cores):
    score[t,s] = <x_t, x_s> - ||x_s||^2/2   (row rank == -dist rank)
    idx[t,:]   = top-8 largest score (self first, descending)
    y[t,(j,o)] = sum_c x[c,t] w[o,c,j] + b[o]/8
    host: out[o,t] = sum_j y[idx[t,j], (j,o)]

Scores need fp32-grade precision (bf16/tf32 rank flips blow the 2e-2
budget). fp16 hi/lo 3-term split gets there at bf16 speed: x = xh + xl
(both fp16), and per 512-chunk two accumulating matmuls compute
  hh + norm-rows   (lhsT [xh;1;1],  rhs [xh;-n/2 hi;-n/2 lo],  K=66)
  hl + lh          (lhsT [xh;xl],   rhs [xl;xh],               K=128)
dropping only xl*xl (~2^-22). Top-8: DVE max8 + find_index8 straight on
the 4-bank PSUM span (fp32, exact). Conv taps are one fp16 matmul per
tile recycling the same PSUM ring after the score tiles. The final
rank-indexed gather+sum runs on the host (no per-partition gather
exists on this hardware: gpsimd indirect_copy shares one index list
per 16-partition group, and DynamicAP indirect DMA is broken).
"""

import sys
import numpy as np

if "/opt/trn_rl_repo" not in sys.path:
    sys.path.insert(0, "/opt/trn_rl_repo")

B, C, T, K, OUT_C = 16, 64, 2048, 8, 64
NCORES = 8
BPC = B // NCORES
RT = T // 128   # 16 row tiles
NF = T // 512   # 4 column chunks

_CACHE = {}


ENABLE_LDW_OPT = False  # walrus crashes with ldw-opt enabled


def _patch_ldw_opt():
    """Let walrus dedupe back-to-back LDWEIGHTS with identical weights
    (concourse hardcodes --enable-ldw-opt=false)."""
    from concourse import bass_utils as _bu

    if getattr(_bu, "_ldwopt_patched", False):
        return
    orig = _bu.run_command

    def run_command_ldwopt(cmd, **kw):
        cmd = [
            "--enable-ldw-opt=true" if c == "--enable-ldw-opt=false" else c
            for c in cmd
        ]
        return orig(cmd, **kw)

    _bu.run_command = run_command_ldwopt
    _bu._ldwopt_patched = True


def build_nc():
    import concourse.bacc as bacc
    import concourse.tile as tile
    import concourse.mybir as mybir

    if ENABLE_LDW_OPT:
        _patch_ldw_opt()

    dt = mybir.dt
    f32 = dt.float32
    f16 = dt.float16

    nc = bacc.Bacc(
        "TRN2", target_bir_lowering=False, debug=False, num_devices=NCORES
    )
    xa_d = nc.dram_tensor("xa", [BPC, 66, T], f16, kind="ExternalInput").ap()
    xb_d = nc.dram_tensor("xb", [BPC, 128, T], f16, kind="ExternalInput").ap()
    ra_d = nc.dram_tensor("ra", [BPC, 66, T], f16, kind="ExternalInput").ap()
    rb_d = nc.dram_tensor("rb", [BPC, 128, T], f16, kind="ExternalInput").ap()
    wall_d = nc.dram_tensor("wall", [65, K * OUT_C], f16, kind="ExternalInput").ap()

    idx_d = nc.dram_tensor("idxout", [BPC, RT, 128, 8], dt.uint16, kind="ExternalOutput").ap()
    y_d = nc.dram_tensor("yout", [BPC, RT, 128, K * OUT_C], f16, kind="ExternalOutput").ap()

    with tile.TileContext(nc) as tc:
        with (
            tc.tile_pool(name="const", bufs=1) as constp,
            tc.tile_pool(name="xio", bufs=1) as xio,
            tc.tile_pool(name="smll", bufs=6) as smll,
            tc.tile_pool(name="yio", bufs=3) as yio,
            tc.tile_pool(name="psc", bufs=2, space="PSUM") as psc,
        ):
            wall_sb = constp.tile([65, K * OUT_C], f16)
            nc.sync.dma_start(wall_sb[:], wall_d[:])

            # input loads: sync and scalar engines generate DMA descriptors
            # in parallel (both are HWDGE-capable) to shorten startup
            xa, xbt, ra, rb = [], [], [], []
            for b in range(BPC):
                xa.append(xio.tile([66, T], f16, name=f"xa{b}"))
                ra.append(xio.tile([66, T], f16, name=f"ra{b}"))
                xbt.append(xio.tile([128, T], f16, name=f"xbt{b}"))
                rb.append(xio.tile([128, T], f16, name=f"rb{b}"))
            for b in range(BPC):
                nc.sync.dma_start(xa[b][:], xa_d[b])
                nc.sync.dma_start(ra[b][:], ra_d[b])
                nc.sync.dma_start(xbt[b][:], xb_d[b])
                nc.sync.dma_start(rb[b][:], rb_d[b])

            # scores + top-8 (DVE scans PSUM directly). Same-weights matmuls
            # are grouped (4x mm1 then 4x mm2) and each tile's conv matmul is
            # interleaved here so the PE never idles waiting on the sc ring.
            for b in range(BPC):
                for rt in range(RT):
                    ts = slice(rt * 128, (rt + 1) * 128)
                    sc = psc.tile([128, T], f32, tag="sc", name=f"sc{b}_{rt}")
                    for nf in range(NF):
                        cs = slice(nf * 512, (nf + 1) * 512)
                        nc.tensor.matmul(
                            sc[:, cs], xa[b][:, ts], ra[b][:, cs],
                            start=True, stop=False,
                        )
                    for nf in range(NF):
                        cs = slice(nf * 512, (nf + 1) * 512)
                        nc.tensor.matmul(
                            sc[:, cs], xbt[b][:, ts], rb[b][:, cs],
                            start=False, stop=True,
                        )
                    gv = smll.tile([128, 8], f32, tag="gv", name=f"gv{b}_{rt}")
                    nc.vector.max(gv[:], sc[:])
                    gidx = smll.tile([128, 8], dt.uint16, tag="gx", name=f"gx{b}_{rt}")
                    nc.vector.max_index(gidx[:], gv[:], sc[:])
                    nc.sync.dma_start(idx_d[b, rt], gidx[:])

                    py = psc.tile([128, T], f32, tag="sc", name=f"py{b}_{rt}")
                    nc.tensor.matmul(py[:, 0 : K * OUT_C], xa[b][0:65, ts], wall_sb[:])
                    ysb = yio.tile([128, K * OUT_C], f16, tag="ysb", name=f"y{b}_{rt}")
                    nc.scalar.copy(ysb[:], py[:, 0 : K * OUT_C])
                    nc.scalar.dma_start(y_d[b, rt], ysb[:])

    nc.compile()
    return nc


def _get_nc():
    if "nc" not in _CACHE:
        _CACHE["nc"] = build_nc()
    return _CACHE["nc"]


def host_inputs(x, w, b):
    f16 = np.float16
    x = np.asarray(x, dtype=np.float32)
    w = np.asarray(w, dtype=np.float32)
    b = np.asarray(b, dtype=np.float32)

    wall = np.zeros((65, K * OUT_C), f16)
    wall[:C] = w.transpose(1, 2, 0).reshape(C, K * OUT_C).astype(f16)  # [c, j*64+o]
    wall[C] = np.tile((b / K).astype(f16), K)

    in_maps = []
    for i in range(NCORES):
        xs = x[i * BPC : (i + 1) * BPC]          # [BPC, C, T]
        xh = xs.astype(f16)
        xl = (xs - xh.astype(np.float32)).astype(f16)
        norm = (xs.astype(np.float64) ** 2).sum(1)          # [BPC, T]
        mh = (-norm / 2).astype(f16)
        ml = (-norm / 2 - mh.astype(np.float64)).astype(f16)

        xa = np.zeros((BPC, 66, T), f16)
        xa[:, :C] = xh
        xa[:, C] = 1.0
        xa[:, C + 1] = 1.0
        ra = np.zeros((BPC, 66, T), f16)
        ra[:, :C] = xh
        ra[:, C] = mh
        ra[:, C + 1] = ml
        xb = np.zeros((BPC, 128, T), f16)
        xb[:, :C] = xh
        xb[:, C:] = xl
        rb = np.zeros((BPC, 128, T), f16)
        rb[:, :C] = xl
        rb[:, C:] = xh
        in_maps.append({"xa": xa, "xb": xb, "ra": ra, "rb": rb, "wall": wall})
    return in_maps


def kernel(x, w, b):
    from concourse.bass_utils import run_bass_kernel_spmd

    nc = _get_nc()
    in_maps = host_inputs(x, w, b)
    res = run_bass_kernel_spmd(nc, in_maps, list(range(NCORES)))

    out = np.empty((B, OUT_C, T), np.float32)
    jj = np.arange(K, dtype=np.int64)[None, :]
    for i in range(NCORES):
        r = res.results[i]
        yv = r["yout"].astype(np.float32).reshape(BPC, T, K, OUT_C)  # [b, t, j, o]
        gidx = r["idxout"].astype(np.int64).reshape(BPC, T, 8)
        for bb in range(BPC):
            idx = gidx[bb]                         # [T, 8] global token ids
            gathered = yv[bb][idx, jj, :]          # [T, K, OUT_C]
            out[i * BPC + bb] = gathered.sum(1).T
    return out.astype(np.float32)
